# revision 9
# baseline (speedup 1.0000x reference)
"""Trainium2 Bass kernel for a 2-layer LSTM (H=64) + FC head.

Problem: x [4096, 168, 19] f32 -> out [4096] f32
  h1 = LSTM0(x); h2 = LSTM1(h1); out = h2[:, -1, :] @ Wfc.T + bfc

Data-parallel over 8 NeuronCores (512 batch rows each). On each core
the batch is further split into CH=2 independent 256-row chains whose
serial recurrences interleave on the engines (latency hiding). Layer 0
at time w and layer 1 at time w-1 are computed together in one "wave"
so every element-wise op uses all 128 partitions:

  PSUM z-tile [128, 4banks, CB]: banks = G, F, I, O gates; partitions
  p0:64 = layer0@w, p64:128 = layer1@{w-1}.
  3 matmuls per bank (fp32r, N=256):
    bias1: b1.T @ ones              (K=1, off critical path)
    L1:    [Wih1; Whh1].T @ hm      (hm = [h0@w-1; h1@w-2], K=128)
    L0:    [Whh0; Wih0; b0].T @ combo   (combo = [h0; x_w; 1], K=84)
  ACT: tanh(G), sigmoid over F,I,O in ONE op (FD=3*CB), tanh(c').
  DVE: u = si*sg; v = sf*c; c' = u+v; hm' = so*tanh(c');
       copy h0 into combo'.
"""

import numpy as np

HIDDEN = 64
INPUT = 19
B = 4096
T = 168
NCORES = 8
BL = B // NCORES   # 512 per core
CH = 2             # chains per core
CB = BL // CH      # 256 per chain
H4 = 4 * HIDDEN    # 256
KC = HIDDEN + INPUT + 1  # 84: combo contraction dim

# torch gate order rows: i(0:64) f(64:128) g(128:192) o(192:256)
# our bank (column-block) order: G, F, I, O
GATE_PERM = np.concatenate([
    np.arange(128, 192),  # g
    np.arange(64, 128),   # f
    np.arange(0, 64),     # i
    np.arange(192, 256),  # o
])


def build_nc(steps=T, fp32r=True):
    import concourse.bacc as bacc
    import concourse.tile as tile
    from concourse import mybir

    F32 = mybir.dt.float32
    FMM = mybir.dt.float32r if fp32r else F32
    AF = mybir.ActivationFunctionType

    nc = bacc.Bacc("TRN2", target_bir_lowering=False, debug=False,
                   num_devices=NCORES)

    xT = nc.dram_tensor("xT", [T, INPUT + 1, BL], FMM, kind="ExternalInput")
    w0big_d = nc.dram_tensor("w0big", [KC, 512], FMM, kind="ExternalInput")
    w1big_d = nc.dram_tensor("w1big", [128, 512], FMM, kind="ExternalInput")
    wfc_d = nc.dram_tensor("wfc", [128, 1], FMM, kind="ExternalInput")
    zeros_d = nc.dram_tensor("zeros", [128, CB], FMM, kind="ExternalInput")
    out = nc.dram_tensor("out", [1, BL], F32, kind="ExternalOutput")

    with tile.TileContext(nc) as tc:
        with (
            tc.tile_pool(name="const", bufs=1) as const,
            tc.tile_pool(name="state", bufs=1) as state,
            tc.tile_pool(name="work", bufs=4) as work,
            tc.tile_pool(name="zpool", bufs=2 * CH, space="PSUM") as zpool,
        ):
            w0big = const.tile([KC, 4, 128], FMM, tag="w0", name="w0")
            w1big = const.tile([128, 4, 128], FMM, tag="w1", name="w1")
            wfc = const.tile([128, 1], FMM, tag="wfc", name="wfc")
            nc.sync.dma_start(w0big, w0big_d[:])
            nc.sync.dma_start(w1big, w1big_d[:])
            nc.sync.dma_start(wfc, wfc_d[:])

            # per-chain state: C = [c0; c1], hm = [h0; h1], combo = [h0; x; 1]
            C = [[state.tile([128, CB], F32, tag=f"C{c}{p}", name=f"C{c}{p}")
                  for p in (0, 1)] for c in range(CH)]
            hm = [[state.tile([128, CB], FMM, tag=f"hm{c}{p}", name=f"hm{c}{p}")
                   for p in (0, 1)] for c in range(CH)]
            combo = [[state.tile([KC, CB], FMM, tag=f"cb{c}{p}", name=f"cb{c}{p}")
                      for p in (0, 1)] for c in range(CH)]
            for c in range(CH):
                cs = slice(c * CB, (c + 1) * CB)
                nc.vector.memset(C[c][0], 0.0)
                nc.sync.dma_start(hm[c][0], zeros_d[:])
                nc.sync.dma_start(combo[c][0][0:64], zeros_d[0:64])
                nc.sync.dma_start(combo[c][0][64:KC], xT[0, :, cs])

            nwaves = steps + 1
            for w in range(nwaves):
                cur, nxt = w % 2, (w + 1) % 2
                for c in range(CH):
                    cs = slice(c * CB, (c + 1) * CB)
                    nc.sync.dma_start(combo[c][nxt][64:KC],
                                      xT[min(w + 1, steps - 1), :, cs])

                    z = zpool.tile([128, 4, CB], F32, tag="z", name=f"z{c}")
                    # per gate-slot: two normal-mode M=128 matmuls.
                    # mm1 (lhsT cols 0:64 = layer-0 gate weights, 64:128 =
                    # b1 on the ones row) initializes the slot; mm2 (lhsT
                    # cols 0:64 = zeros, 64:128 = [Wih1; Whh1]) accumulates
                    # layer-1 on top from hm = [h0; h1]. Full-partition M=128
                    # avoids column tiling, which fp32r weights can't use.
                    # Groups must not interleave within a HW PSUM bank.
                    for b in range(4):
                        nc.tensor.matmul(z[:, b, :], w0big[:, b, :],
                                         combo[c][cur][:], start=True,
                                         stop=False, skip_group_check=True)
                        nc.tensor.matmul(z[:, b, :], w1big[:, b, :],
                                         hm[c][cur][:], start=False,
                                         stop=True, skip_group_check=True)

                    zf = z.rearrange("p b n -> p (b n)")
                    sg = work.tile([128, CB], F32, tag=f"sg{c}", name=f"sg{c}")
                    nc.scalar.activation(sg, zf[:, 0:CB], AF.Tanh)
                    s_fio = work.tile([128, 3 * CB], F32, tag=f"sfio{c}",
                                      name=f"sfio{c}")
                    nc.scalar.activation(s_fio, zf[:, CB:4 * CB], AF.Sigmoid)

                    sf = s_fio[:, 0:CB]
                    si = s_fio[:, CB:2 * CB]
                    so = s_fio[:, 2 * CB:3 * CB]
                    u = work.tile([128, CB], F32, tag=f"u{c}", name=f"u{c}")
                    nc.vector.tensor_mul(u, si, sg)
                    v = work.tile([128, CB], F32, tag=f"v{c}", name=f"v{c}")
                    nc.gpsimd.tensor_mul(v, sf, C[c][cur])
                    nc.vector.tensor_add(C[c][nxt], u, v)

                    stc = work.tile([128, CB], F32, tag=f"stc{c}",
                                    name=f"stc{c}")
                    nc.scalar.activation(stc, C[c][nxt], AF.Tanh)
                    nc.vector.tensor_mul(hm[c][nxt], so, stc)
                    nc.gpsimd.tensor_copy(combo[c][nxt][0:64],
                                          hm[c][nxt][0:64])

                    if w == 0:
                        # wave 0's layer-1 half ran on garbage; reset it
                        nc.vector.memset(C[c][nxt][64:128], 0.0)
                        nc.sync.dma_start(hm[c][nxt][64:128],
                                          zeros_d[64:128])

            # --- FC head: out = Wfc . h1@steps-1 (bfc added on host) ---
            o_sb = work.tile([1, BL], F32, tag="osb", name="o_sb")
            for c in range(CH):
                pfc = zpool.tile([1, CB], F32, tag="z", name=f"pfc{c}")
                nc.tensor.matmul(pfc, wfc, hm[c][nwaves % 2][:],
                                 start=True, stop=True)
                nc.scalar.activation(o_sb[:, c * CB:(c + 1) * CB], pfc,
                                     AF.Copy)
            nc.sync.dma_start(out[:], o_sb)

    nc.compile()
    return nc


def make_in_maps(x, Wih0, Whh0, bih0, bhh0, Wih1, Whh1, bih1, bhh1, Wfc, bfc):
    """Shard + pre-transpose/concat inputs for the 8 cores."""
    p = GATE_PERM
    b0 = (bih0 + bhh0)[p].astype(np.float32)
    b1 = (bih1 + bhh1)[p].astype(np.float32)
    w0cat = np.concatenate([
        Whh0[p].T,            # [64, 256]
        Wih0[p].T,            # [19, 256]
        b0.reshape(1, H4),    # [1, 256]
    ], axis=0).astype(np.float32)
    # w0big [KC, 4 banks, 128]: left 64 cols = layer0 gate weights,
    # right 64 cols = b1 on the ones row (bias for the layer1 half)
    w0big = np.zeros((KC, 4, 128), np.float32)
    for b in range(4):
        w0big[:, b, 0:64] = w0cat[:, b * 64:(b + 1) * 64]
        w0big[KC - 1, b, 64:128] = b1[b * 64:(b + 1) * 64]
    w1cat = np.concatenate([
        Wih1[p].T,            # [64, 256]
        Whh1[p].T,            # [64, 256]
    ], axis=0).astype(np.float32)
    # w1big [128, 4, 128]: left 64 cols zero (layer-0 half untouched),
    # right 64 cols = layer-1 gate weights; rhs is hm = [h0; h1]
    w1big = np.zeros((128, 4, 128), np.float32)
    for b in range(4):
        w1big[:, b, 64:128] = w1cat[:, b * 64:(b + 1) * 64]
    wfcbig = np.zeros((128, 1), np.float32)
    wfcbig[64:128, 0] = Wfc.reshape(HIDDEN)
    base = {
        "w0big": np.ascontiguousarray(w0big.reshape(KC, 512)),
        "w1big": np.ascontiguousarray(w1big.reshape(128, 512)),
        "wfc": wfcbig,
        "zeros": np.zeros((128, CB), np.float32),
    }
    xs = x.reshape(NCORES, BL, T, INPUT)
    in_maps = []
    for c in range(NCORES):
        m = dict(base)
        xt = np.empty((T, INPUT + 1, BL), np.float32)
        xt[:, 0:INPUT, :] = xs[c].transpose(1, 2, 0)
        xt[:, INPUT, :] = 1.0
        m["xT"] = xt
        in_maps.append(m)
    return in_maps


_CACHED_NC = None


def kernel(**inputs):
    global _CACHED_NC
    from concourse.bass_utils import run_bass_kernel_spmd

    if _CACHED_NC is None:
        _CACHED_NC = build_nc()
    nc = _CACHED_NC
    in_maps = make_in_maps(**inputs)
    res = run_bass_kernel_spmd(nc, in_maps, list(range(NCORES)))
    outs = [res.results[c]["out"].reshape(BL) for c in range(NCORES)]
    return np.concatenate(outs) + np.float32(inputs["bfc"][0])


# revision 10
# speedup vs baseline: 2.7131x; 2.7131x over previous
"""Trainium2 Bass kernel for a 2-layer LSTM (H=64) + FC head.

Problem: x [4096, 168, 19] f32 -> out [4096] f32
  h1 = LSTM0(x); h2 = LSTM1(h1); out = h2[:, -1, :] @ Wfc.T + bfc

Data-parallel over 8 NeuronCores (512 batch rows each). On each core
the batch is further split into CH=2 independent 256-row chains whose
serial recurrences interleave on the engines (latency hiding). Layer 0
at time w and layer 1 at time w-1 are computed together in one "wave"
so every element-wise op uses all 128 partitions:

  PSUM z-tile [128, 4banks, CB]: banks = G, F, I, O gates; partitions
  p0:64 = layer0@w, p64:128 = layer1@{w-1}.
  3 matmuls per bank (fp32r, N=256):
    bias1: b1.T @ ones              (K=1, off critical path)
    L1:    [Wih1; Whh1].T @ hm      (hm = [h0@w-1; h1@w-2], K=128)
    L0:    [Whh0; Wih0; b0].T @ combo   (combo = [h0; x_w; 1], K=84)
  ACT: tanh(G), sigmoid over F,I,O in ONE op (FD=3*CB), tanh(c').
  DVE: u = si*sg; v = sf*c; c' = u+v; hm' = so*tanh(c');
       copy h0 into combo'.
"""

import numpy as np

HIDDEN = 64
INPUT = 19
B = 4096
T = 168
NCORES = 8
BL = B // NCORES   # 512 per core
CH = 2             # chains per core
CB = BL // CH      # 256 per chain
H4 = 4 * HIDDEN    # 256
KC = HIDDEN + INPUT + 1  # 84: combo contraction dim

# torch gate order rows: i(0:64) f(64:128) g(128:192) o(192:256)
# our bank (column-block) order: G, F, I, O
GATE_PERM = np.concatenate([
    np.arange(128, 192),  # g
    np.arange(64, 128),   # f
    np.arange(0, 64),     # i
    np.arange(192, 256),  # o
])


def build_nc(steps=T, fp32r=True):
    import concourse.bacc as bacc
    import concourse.tile as tile
    from concourse import mybir

    F32 = mybir.dt.float32
    FMM = mybir.dt.float32r if fp32r else F32
    AF = mybir.ActivationFunctionType

    nc = bacc.Bacc("TRN2", target_bir_lowering=False, debug=False,
                   num_devices=NCORES)

    xT = nc.dram_tensor("xT", [T, INPUT + 1, BL], FMM, kind="ExternalInput")
    w0big_d = nc.dram_tensor("w0big", [KC, 512], FMM, kind="ExternalInput")
    w1big_d = nc.dram_tensor("w1big", [128, 512], FMM, kind="ExternalInput")
    wfc_d = nc.dram_tensor("wfc", [128, 1], FMM, kind="ExternalInput")
    zeros_d = nc.dram_tensor("zeros", [128, CB], FMM, kind="ExternalInput")
    out = nc.dram_tensor("out", [1, BL], F32, kind="ExternalOutput")

    with tile.TileContext(nc) as tc:
        with (
            tc.tile_pool(name="const", bufs=1) as const,
            tc.tile_pool(name="state", bufs=1) as state,
            tc.tile_pool(name="work", bufs=4) as work,
            tc.tile_pool(name="zpool", bufs=2 * CH, space="PSUM") as zpool,
        ):
            w0big = const.tile([KC, 4, 128], FMM, tag="w0", name="w0")
            w1big = const.tile([128, 4, 128], FMM, tag="w1", name="w1")
            wfc = const.tile([128, 1], FMM, tag="wfc", name="wfc")
            nc.sync.dma_start(w0big, w0big_d[:])
            nc.sync.dma_start(w1big, w1big_d[:])
            nc.sync.dma_start(wfc, wfc_d[:])

            # per-chain state: C = [c0; c1], hm = [h0; h1], combo = [h0; x; 1]
            C = [[state.tile([128, CB], F32, tag=f"C{c}{p}", name=f"C{c}{p}")
                  for p in (0, 1)] for c in range(CH)]
            hm = [[state.tile([128, CB], FMM, tag=f"hm{c}{p}", name=f"hm{c}{p}")
                   for p in (0, 1)] for c in range(CH)]
            combo = [[state.tile([KC, CB], FMM, tag=f"cb{c}{p}", name=f"cb{c}{p}")
                      for p in (0, 1)] for c in range(CH)]
            for c in range(CH):
                cs = slice(c * CB, (c + 1) * CB)
                nc.vector.memset(C[c][0], 0.0)
                nc.sync.dma_start(hm[c][0], zeros_d[:])
                nc.sync.dma_start(combo[c][0][0:64], zeros_d[0:64])
                nc.sync.dma_start(combo[c][0][64:KC], xT[0, :, cs])

            nwaves = steps + 1
            for w in range(nwaves):
                cur, nxt = w % 2, (w + 1) % 2
                for c in range(CH):
                    cs = slice(c * CB, (c + 1) * CB)
                    nc.sync.dma_start(combo[c][nxt][64:KC],
                                      xT[min(w + 1, steps - 1) % T, :, cs])

                    z = zpool.tile([128, 4, CB], F32, tag="z", name=f"z{c}")
                    # per gate-slot: two normal-mode M=128 matmuls.
                    # mm1 (lhsT cols 0:64 = layer-0 gate weights, 64:128 =
                    # b1 on the ones row) initializes the slot; mm2 (lhsT
                    # cols 0:64 = zeros, 64:128 = [Wih1; Whh1]) accumulates
                    # layer-1 on top from hm = [h0; h1]. Full-partition M=128
                    # avoids column tiling, which fp32r weights can't use.
                    # Groups must not interleave within a HW PSUM bank.
                    for b in range(4):
                        nc.tensor.matmul(z[:, b, :], w0big[:, b, :],
                                         combo[c][cur][:], start=True,
                                         stop=False, skip_group_check=True)
                        nc.tensor.matmul(z[:, b, :], w1big[:, b, :],
                                         hm[c][cur][:], start=False,
                                         stop=True, skip_group_check=True)

                    zf = z.rearrange("p b n -> p (b n)")
                    sg = work.tile([128, CB], F32, tag=f"sg{c}", name=f"sg{c}")
                    nc.scalar.activation(sg, zf[:, 0:CB], AF.Tanh)
                    s_fio = work.tile([128, 3 * CB], F32, tag=f"sfio{c}",
                                      name=f"sfio{c}")
                    nc.scalar.activation(s_fio, zf[:, CB:4 * CB], AF.Sigmoid)

                    sf = s_fio[:, 0:CB]
                    si = s_fio[:, CB:2 * CB]
                    so = s_fio[:, 2 * CB:3 * CB]
                    u = work.tile([128, CB], F32, tag=f"u{c}", name=f"u{c}")
                    nc.vector.tensor_mul(u, si, sg)
                    v = work.tile([128, CB], F32, tag=f"v{c}", name=f"v{c}")
                    nc.gpsimd.tensor_mul(v, sf, C[c][cur])
                    nc.vector.tensor_add(C[c][nxt], u, v)

                    stc = work.tile([128, CB], F32, tag=f"stc{c}",
                                    name=f"stc{c}")
                    nc.scalar.activation(stc, C[c][nxt], AF.Tanh)
                    nc.vector.tensor_mul(hm[c][nxt], so, stc)
                    nc.gpsimd.tensor_copy(combo[c][nxt][0:64],
                                          hm[c][nxt][0:64])

                    if w == 0:
                        # wave 0's layer-1 half ran on garbage; reset it
                        nc.vector.memset(C[c][nxt][64:128], 0.0)
                        nc.sync.dma_start(hm[c][nxt][64:128],
                                          zeros_d[64:128])

            # --- FC head: out = Wfc . h1@steps-1 (bfc added on host) ---
            o_sb = work.tile([1, BL], F32, tag="osb", name="o_sb")
            for c in range(CH):
                pfc = zpool.tile([1, CB], F32, tag="z", name=f"pfc{c}")
                nc.tensor.matmul(pfc, wfc, hm[c][nwaves % 2][:],
                                 start=True, stop=True)
                nc.scalar.activation(o_sb[:, c * CB:(c + 1) * CB], pfc,
                                     AF.Copy)
            nc.sync.dma_start(out[:], o_sb)

    nc.compile()
    return nc


def make_in_maps(x, Wih0, Whh0, bih0, bhh0, Wih1, Whh1, bih1, bhh1, Wfc, bfc):
    """Shard + pre-transpose/concat inputs for the 8 cores."""
    p = GATE_PERM
    b0 = (bih0 + bhh0)[p].astype(np.float32)
    b1 = (bih1 + bhh1)[p].astype(np.float32)
    w0cat = np.concatenate([
        Whh0[p].T,            # [64, 256]
        Wih0[p].T,            # [19, 256]
        b0.reshape(1, H4),    # [1, 256]
    ], axis=0).astype(np.float32)
    # w0big [KC, 4 banks, 128]: left 64 cols = layer0 gate weights,
    # right 64 cols = b1 on the ones row (bias for the layer1 half)
    w0big = np.zeros((KC, 4, 128), np.float32)
    for b in range(4):
        w0big[:, b, 0:64] = w0cat[:, b * 64:(b + 1) * 64]
        w0big[KC - 1, b, 64:128] = b1[b * 64:(b + 1) * 64]
    w1cat = np.concatenate([
        Wih1[p].T,            # [64, 256]
        Whh1[p].T,            # [64, 256]
    ], axis=0).astype(np.float32)
    # w1big [128, 4, 128]: left 64 cols zero (layer-0 half untouched),
    # right 64 cols = layer-1 gate weights; rhs is hm = [h0; h1]
    w1big = np.zeros((128, 4, 128), np.float32)
    for b in range(4):
        w1big[:, b, 64:128] = w1cat[:, b * 64:(b + 1) * 64]
    wfcbig = np.zeros((128, 1), np.float32)
    wfcbig[64:128, 0] = Wfc.reshape(HIDDEN)
    base = {
        "w0big": np.ascontiguousarray(w0big.reshape(KC, 512)),
        "w1big": np.ascontiguousarray(w1big.reshape(128, 512)),
        "wfc": wfcbig,
        "zeros": np.zeros((128, CB), np.float32),
    }
    xs = x.reshape(NCORES, BL, T, INPUT)
    in_maps = []
    for c in range(NCORES):
        m = dict(base)
        xt = np.empty((T, INPUT + 1, BL), np.float32)
        xt[:, 0:INPUT, :] = xs[c].transpose(1, 2, 0)
        xt[:, INPUT, :] = 1.0
        m["xT"] = xt
        in_maps.append(m)
    return in_maps


_CACHED_NC = None


def kernel(**inputs):
    global _CACHED_NC
    from concourse.bass_utils import run_bass_kernel_spmd

    if _CACHED_NC is None:
        _CACHED_NC = build_nc()
    nc = _CACHED_NC
    in_maps = make_in_maps(**inputs)
    res = run_bass_kernel_spmd(nc, in_maps, list(range(NCORES)))
    outs = [res.results[c]["out"].reshape(BL) for c in range(NCORES)]
    return np.concatenate(outs) + np.float32(inputs["bfc"][0])


# revision 11
# speedup vs baseline: 3.5221x; 1.2982x over previous
"""Trainium2 Bass kernel for a 2-layer LSTM (H=64) + FC head.

Problem: x [4096, 168, 19] f32 -> out [4096] f32
  h1 = LSTM0(x); h2 = LSTM1(h1); out = h2[:, -1, :] @ Wfc.T + bfc

Data-parallel over 8 NeuronCores (512 batch rows each). On each core
the batch is further split into CH=2 independent 256-row chains whose
serial recurrences interleave on the engines (latency hiding). Layer 0
at time w and layer 1 at time w-1 are computed together in one "wave"
so every element-wise op uses all 128 partitions:

  PSUM z-tile [128, 4banks, CB]: banks = G, F, I, O gates; partitions
  p0:64 = layer0@w, p64:128 = layer1@{w-1}.
  3 matmuls per bank (fp32r, N=256):
    bias1: b1.T @ ones              (K=1, off critical path)
    L1:    [Wih1; Whh1].T @ hm      (hm = [h0@w-1; h1@w-2], K=128)
    L0:    [Whh0; Wih0; b0].T @ combo   (combo = [h0; x_w; 1], K=84)
  ACT: tanh(G), sigmoid over F,I,O in ONE op (FD=3*CB), tanh(c').
  DVE: u = si*sg; v = sf*c; c' = u+v; hm' = so*tanh(c');
       copy h0 into combo'.
"""

import numpy as np

HIDDEN = 64
INPUT = 19
B = 4096
T = 168
NCORES = 8
BL = B // NCORES   # 512 per core
CH = 2             # chains per core
CB = BL // CH      # 256 per chain
H4 = 4 * HIDDEN    # 256
KC = HIDDEN + INPUT + 1  # 84: combo contraction dim

# torch gate order rows: i(0:64) f(64:128) g(128:192) o(192:256)
# our bank (column-block) order: G, F, I, O
GATE_PERM = np.concatenate([
    np.arange(128, 192),  # g
    np.arange(64, 128),   # f
    np.arange(0, 64),     # i
    np.arange(192, 256),  # o
])


def build_nc(steps=T, fp32r=True, repeat=1):
    import concourse.bacc as bacc
    import concourse.tile as tile
    from concourse import mybir

    F32 = mybir.dt.float32
    FMM = mybir.dt.float32r if fp32r else F32
    AF = mybir.ActivationFunctionType

    nc = bacc.Bacc("TRN2", target_bir_lowering=False, debug=False,
                   num_devices=NCORES)

    xT = nc.dram_tensor("xT", [T, INPUT + 1, BL], FMM, kind="ExternalInput")
    w0big_d = nc.dram_tensor("w0big", [KC, 512], FMM, kind="ExternalInput")
    w1big_d = nc.dram_tensor("w1big", [128, 512], FMM, kind="ExternalInput")
    wfc_d = nc.dram_tensor("wfc", [128, 1], FMM, kind="ExternalInput")
    zeros_d = nc.dram_tensor("zeros", [128, CB], FMM, kind="ExternalInput")
    out = nc.dram_tensor("out", [1, BL], F32, kind="ExternalOutput")

    with tile.TileContext(nc) as tc:
        with (
            tc.tile_pool(name="const", bufs=1) as const,
            tc.tile_pool(name="state", bufs=1) as state,
            tc.tile_pool(name="work", bufs=4) as work,
            tc.tile_pool(name="zpool", bufs=2 * CH, space="PSUM") as zpool,
        ):
            w0big = const.tile([KC, 4, 128], FMM, tag="w0", name="w0")
            w1big = const.tile([128, 4, 128], FMM, tag="w1", name="w1")
            wfc = const.tile([128, 1], FMM, tag="wfc", name="wfc")
            nc.sync.dma_start(w0big, w0big_d[:])
            nc.sync.dma_start(w1big, w1big_d[:])
            nc.sync.dma_start(wfc, wfc_d[:])

            # per-chain state: C = [c0; c1], hm = [h0; h1], combo = [h0; x; 1]
            C = [[state.tile([128, CB], F32, tag=f"C{c}{p}", name=f"C{c}{p}")
                  for p in (0, 1)] for c in range(CH)]
            hm = [[state.tile([128, CB], FMM, tag=f"hm{c}{p}", name=f"hm{c}{p}")
                   for p in (0, 1)] for c in range(CH)]
            combo = [[state.tile([KC, CB], FMM, tag=f"cb{c}{p}", name=f"cb{c}{p}")
                      for p in (0, 1)] for c in range(CH)]
            for c in range(CH):
                cs = slice(c * CB, (c + 1) * CB)
                nc.vector.memset(C[c][0], 0.0)
                nc.sync.dma_start(hm[c][0], zeros_d[:])
                nc.sync.dma_start(combo[c][0][0:64], zeros_d[0:64])
                nc.sync.dma_start(combo[c][0][64:KC], xT[0, :, cs])

            nwaves = steps + 1

            def wave_body(w):
                cur, nxt = w % 2, (w + 1) % 2
                for c in range(CH):
                    cs = slice(c * CB, (c + 1) * CB)
                    nc.sync.dma_start(combo[c][nxt][64:KC],
                                      xT[min(w + 1, steps - 1) % T, :, cs])

                    z = zpool.tile([128, 4, CB], F32, tag="z", name=f"z{c}")
                    # per gate-slot: two normal-mode M=128 matmuls.
                    # mm1 (lhsT cols 0:64 = layer-0 gate weights, 64:128 =
                    # b1 on the ones row) initializes the slot; mm2 (lhsT
                    # cols 0:64 = zeros, 64:128 = [Wih1; Whh1]) accumulates
                    # layer-1 on top from hm = [h0; h1]. Full-partition M=128
                    # avoids column tiling, which fp32r weights can't use.
                    # Groups must not interleave within a HW PSUM bank.
                    for b in range(4):
                        nc.tensor.matmul(z[:, b, :], w0big[:, b, :],
                                         combo[c][cur][:], start=True,
                                         stop=False, skip_group_check=True)
                        nc.tensor.matmul(z[:, b, :], w1big[:, b, :],
                                         hm[c][cur][:], start=False,
                                         stop=True, skip_group_check=True)

                    zf = z.rearrange("p b n -> p (b n)")
                    sg = work.tile([128, CB], F32, tag=f"sg{c}", name=f"sg{c}")
                    nc.scalar.activation(sg, zf[:, 0:CB], AF.Tanh)
                    s_fio = work.tile([128, 3 * CB], F32, tag=f"sfio{c}",
                                      name=f"sfio{c}")
                    nc.scalar.activation(s_fio, zf[:, CB:4 * CB], AF.Sigmoid)

                    sf = s_fio[:, 0:CB]
                    si = s_fio[:, CB:2 * CB]
                    so = s_fio[:, 2 * CB:3 * CB]
                    u = work.tile([128, CB], F32, tag=f"u{c}", name=f"u{c}")
                    nc.vector.tensor_mul(u, si, sg)
                    v = work.tile([128, CB], F32, tag=f"v{c}", name=f"v{c}")
                    nc.gpsimd.tensor_mul(v, sf, C[c][cur])
                    nc.vector.tensor_add(C[c][nxt], u, v)

                    stc = work.tile([128, CB], F32, tag=f"stc{c}",
                                    name=f"stc{c}")
                    nc.scalar.activation(stc, C[c][nxt], AF.Tanh)
                    nc.vector.tensor_mul(hm[c][nxt], so, stc)
                    nc.gpsimd.tensor_copy(combo[c][nxt][0:64],
                                          hm[c][nxt][0:64])

                    if w == 0:
                        # wave 0's layer-1 half ran on garbage; reset it
                        nc.vector.memset(C[c][nxt][64:128], 0.0)
                        nc.sync.dma_start(hm[c][nxt][64:128],
                                          zeros_d[64:128])

            if repeat == 1:
                for w in range(nwaves):
                    wave_body(w)
            else:
                import concourse.tile as _tile
                def rep_body(_i):
                    for w in range(nwaves):
                        wave_body(w)
                with tc.For_i(0, repeat, 1) as _i:
                    rep_body(_i)

            # --- FC head: out = Wfc . h1@steps-1 (bfc added on host) ---
            o_sb = work.tile([1, BL], F32, tag="osb", name="o_sb")
            for c in range(CH):
                pfc = zpool.tile([1, CB], F32, tag="z", name=f"pfc{c}")
                nc.tensor.matmul(pfc, wfc, hm[c][nwaves % 2][:],
                                 start=True, stop=True)
                nc.scalar.activation(o_sb[:, c * CB:(c + 1) * CB], pfc,
                                     AF.Copy)
            nc.sync.dma_start(out[:], o_sb)

    nc.compile()
    return nc


def make_in_maps(x, Wih0, Whh0, bih0, bhh0, Wih1, Whh1, bih1, bhh1, Wfc, bfc):
    """Shard + pre-transpose/concat inputs for the 8 cores."""
    p = GATE_PERM
    b0 = (bih0 + bhh0)[p].astype(np.float32)
    b1 = (bih1 + bhh1)[p].astype(np.float32)
    w0cat = np.concatenate([
        Whh0[p].T,            # [64, 256]
        Wih0[p].T,            # [19, 256]
        b0.reshape(1, H4),    # [1, 256]
    ], axis=0).astype(np.float32)
    # w0big [KC, 4 banks, 128]: left 64 cols = layer0 gate weights,
    # right 64 cols = b1 on the ones row (bias for the layer1 half)
    w0big = np.zeros((KC, 4, 128), np.float32)
    for b in range(4):
        w0big[:, b, 0:64] = w0cat[:, b * 64:(b + 1) * 64]
        w0big[KC - 1, b, 64:128] = b1[b * 64:(b + 1) * 64]
    w1cat = np.concatenate([
        Wih1[p].T,            # [64, 256]
        Whh1[p].T,            # [64, 256]
    ], axis=0).astype(np.float32)
    # w1big [128, 4, 128]: left 64 cols zero (layer-0 half untouched),
    # right 64 cols = layer-1 gate weights; rhs is hm = [h0; h1]
    w1big = np.zeros((128, 4, 128), np.float32)
    for b in range(4):
        w1big[:, b, 64:128] = w1cat[:, b * 64:(b + 1) * 64]
    wfcbig = np.zeros((128, 1), np.float32)
    wfcbig[64:128, 0] = Wfc.reshape(HIDDEN)
    base = {
        "w0big": np.ascontiguousarray(w0big.reshape(KC, 512)),
        "w1big": np.ascontiguousarray(w1big.reshape(128, 512)),
        "wfc": wfcbig,
        "zeros": np.zeros((128, CB), np.float32),
    }
    xs = x.reshape(NCORES, BL, T, INPUT)
    in_maps = []
    for c in range(NCORES):
        m = dict(base)
        xt = np.empty((T, INPUT + 1, BL), np.float32)
        xt[:, 0:INPUT, :] = xs[c].transpose(1, 2, 0)
        xt[:, INPUT, :] = 1.0
        m["xT"] = xt
        in_maps.append(m)
    return in_maps


_CACHED_NC = None


def kernel(**inputs):
    global _CACHED_NC
    from concourse.bass_utils import run_bass_kernel_spmd

    if _CACHED_NC is None:
        _CACHED_NC = build_nc()
    nc = _CACHED_NC
    in_maps = make_in_maps(**inputs)
    res = run_bass_kernel_spmd(nc, in_maps, list(range(NCORES)))
    outs = [res.results[c]["out"].reshape(BL) for c in range(NCORES)]
    return np.concatenate(outs) + np.float32(inputs["bfc"][0])


# revision 23
# speedup vs baseline: 12.6253x; 3.5845x over previous
"""Trainium2 Bass kernel for a 2-layer LSTM (H=64) + FC head.

Problem: x [4096, 168, 19] f32 -> out [4096] f32
  h1 = LSTM0(x); h2 = LSTM1(h1); out = h2[:, -1, :] @ Wfc.T + bfc

Data-parallel over 8 NeuronCores (512 batch rows each). On each core
the batch is further split into CH=2 independent 256-row chains whose
serial recurrences interleave on the engines (latency hiding). Layer 0
at time w and layer 1 at time w-1 are computed together in one "wave"
so every element-wise op uses all 128 partitions:

  PSUM z-tile [128, 4banks, CB]: banks = G, F, I, O gates; partitions
  p0:64 = layer0@w, p64:128 = layer1@{w-1}.
  3 matmuls per bank (fp32r, N=256):
    bias1: b1.T @ ones              (K=1, off critical path)
    L1:    [Wih1; Whh1].T @ hm      (hm = [h0@w-1; h1@w-2], K=128)
    L0:    [Whh0; Wih0; b0].T @ combo   (combo = [h0; x_w; 1], K=84)
  ACT: tanh(G), sigmoid over F,I,O in ONE op (FD=3*CB), tanh(c').
  DVE: u = si*sg; v = sf*c; c' = u+v; hm' = so*tanh(c');
       copy h0 into combo'.
"""

import numpy as np

HIDDEN = 64
INPUT = 19
B = 4096
T = 168
NCORES = 8
BL = B // NCORES   # 512 per core
CH = 2             # chains per core
CB = BL // CH      # 256 per chain
H4 = 4 * HIDDEN    # 256
KC = HIDDEN + INPUT + 1  # 84: combo contraction dim

# torch gate order rows: i(0:64) f(64:128) g(128:192) o(192:256)
# our bank (column-block) order: G, F, I, O
GATE_PERM = np.concatenate([
    np.arange(128, 192),  # g
    np.arange(64, 128),   # f
    np.arange(0, 64),     # i
    np.arange(192, 256),  # o
])


def build_nc(steps=T, fp32r=True, repeat=1):
    import concourse.bacc as bacc
    import concourse.tile as tile
    from concourse import mybir

    F32 = mybir.dt.float32
    FMM = mybir.dt.float32r if fp32r else F32
    AF = mybir.ActivationFunctionType

    nc = bacc.Bacc("TRN2", target_bir_lowering=False, debug=False,
                   num_devices=NCORES)

    xT = nc.dram_tensor("xT", [T, INPUT + 1, BL], FMM, kind="ExternalInput")
    w0x_d = nc.dram_tensor("w0x", [INPUT + 1, 512], FMM, kind="ExternalInput")
    whbig_d = nc.dram_tensor("whbig", [128, 512], FMM, kind="ExternalInput")
    wfc_d = nc.dram_tensor("wfc", [128, 1], FMM, kind="ExternalInput")
    zeros_d = nc.dram_tensor("zeros", [128, CB], FMM, kind="ExternalInput")
    out = nc.dram_tensor("out", [1, BL], F32, kind="ExternalOutput")

    with tile.TileContext(nc) as tc:
        with (
            tc.tile_pool(name="const", bufs=1) as const,
            tc.tile_pool(name="state", bufs=1) as state,
            tc.tile_pool(name="work", bufs=6) as work,
            tc.tile_pool(name="xin", bufs=6) as xin,
            tc.tile_pool(name="zpool", bufs=2 * CH, space="PSUM") as zpool,
        ):
            w0x = const.tile([INPUT + 1, 4, 128], FMM, tag="w0x", name="w0x")
            whbig = const.tile([128, 4, 128], FMM, tag="wh", name="whbig")
            wfc = const.tile([128, 1], FMM, tag="wfc", name="wfc")
            nc.sync.dma_start(w0x, w0x_d[:])
            nc.sync.dma_start(whbig, whbig_d[:])
            nc.sync.dma_start(wfc, wfc_d[:])

            # per-chain state: C = [c0; c1], hm = [h0; h1]
            C = [[state.tile([128, CB], F32, tag=f"C{c}{p}", name=f"C{c}{p}")
                  for p in (0, 1)] for c in range(CH)]
            hm = [[state.tile([128, CB], FMM, tag=f"hm{c}{p}", name=f"hm{c}{p}")
                   for p in (0, 1)] for c in range(CH)]
            for c in range(CH):
                nc.vector.memset(C[c][0], 0.0)
                nc.sync.dma_start(hm[c][0], zeros_d[:])

            nwaves = steps + 1

            def wave_body(w):
                cur, nxt = w % 2, (w + 1) % 2
                # Phase-interleaved emission: engines are in-order, so
                # chain B's sigmoid must sit between chain A's sigmoid and
                # A's tanh(c') in the static ACT order to fill the gap
                # while A's DVE ops run (and vice versa on DVE).
                zs, sgs, sfios, stcs = [], [], [], []
                for c in range(CH):
                    cs = slice(c * CB, (c + 1) * CB)
                    xt = xin.tile([INPUT + 1, CB], FMM, tag=f"x{c}",
                                  name=f"x{c}")
                    nc.sync.dma_start(xt, xT[w % T, :, cs])

                    z = zpool.tile([128, 4, CB], F32, tag="z", name=f"z{c}")
                    zs.append(z)
                    # per gate-slot: two normal-mode M=128 matmuls.
                    # mm1 (x-part, off the recurrence critical path: lhsT
                    # cols 0:64 = [Wih0; b0], cols 64:128 = b1 on the ones
                    # row) initializes the slot; mm2 (K=128, lhsT left
                    # cols = [Whh0; 0], right cols = [Wih1; Whh1]) adds
                    # BOTH layers' h-contributions from hm = [h0; h1] in
                    # one shot. Full-partition M=128 avoids column tiling,
                    # which fp32r weights can't use. Groups must not
                    # interleave within a HW PSUM bank.
                    for b in range(4):
                        nc.tensor.matmul(z[:, b, :], w0x[:, b, :],
                                         xt[:], start=True,
                                         stop=False, skip_group_check=True)
                        nc.tensor.matmul(z[:, b, :], whbig[:, b, :],
                                         hm[c][cur][:], start=False,
                                         stop=True, skip_group_check=True)

                    zf = z.rearrange("p b n -> p (b n)")
                    sg = work.tile([128, CB], F32, tag=f"sg{c}", name=f"sg{c}")
                    nc.scalar.activation(sg, zf[:, 0:CB], AF.Tanh)
                    sgs.append(sg)
                    s_fio = work.tile([128, 3 * CB], F32, tag=f"sfio{c}",
                                      name=f"sfio{c}")
                    nc.scalar.activation(s_fio, zf[:, CB:4 * CB], AF.Sigmoid)
                    sfios.append(s_fio)

                for c in range(CH):
                    s_fio, sg = sfios[c], sgs[c]
                    sf = s_fio[:, 0:CB]
                    si = s_fio[:, CB:2 * CB]
                    u = work.tile([128, CB], F32, tag=f"u{c}", name=f"u{c}")
                    nc.vector.tensor_mul(u, si, sg)
                    v = work.tile([128, CB], F32, tag=f"v{c}", name=f"v{c}")
                    nc.gpsimd.tensor_mul(v, sf, C[c][cur])
                    nc.vector.tensor_add(C[c][nxt], u, v)

                for c in range(CH):
                    stc = work.tile([128, CB], F32, tag=f"stc{c}",
                                    name=f"stc{c}")
                    nc.scalar.activation(stc, C[c][nxt], AF.Tanh)
                    stcs.append(stc)

                for c in range(CH):
                    so = sfios[c][:, 2 * CB:3 * CB]
                    nc.vector.tensor_mul(hm[c][nxt], so, stcs[c])

                if w == 0:
                    # wave 0's layer-1 half ran on garbage; reset it
                    for c in range(CH):
                        nc.vector.memset(C[c][nxt][64:128], 0.0)
                        nc.sync.dma_start(hm[c][nxt][64:128],
                                          zeros_d[64:128])

            if repeat == 1:
                for w in range(nwaves):
                    wave_body(w)
            else:
                import concourse.tile as _tile
                def rep_body(_i):
                    for w in range(nwaves):
                        wave_body(w)
                with tc.For_i(0, repeat, 1) as _i:
                    rep_body(_i)

            # --- FC head: out = Wfc . h1@steps-1 (bfc added on host) ---
            o_sb = work.tile([1, BL], F32, tag="osb", name="o_sb")
            for c in range(CH):
                pfc = zpool.tile([1, CB], F32, tag="z", name=f"pfc{c}")
                nc.tensor.matmul(pfc, wfc, hm[c][nwaves % 2][:],
                                 start=True, stop=True)
                nc.scalar.activation(o_sb[:, c * CB:(c + 1) * CB], pfc,
                                     AF.Copy)
            nc.sync.dma_start(out[:], o_sb)

    nc.compile()
    return nc


def make_in_maps(x, Wih0, Whh0, bih0, bhh0, Wih1, Whh1, bih1, bhh1, Wfc, bfc):
    """Shard + pre-transpose/concat inputs for the 8 cores."""
    p = GATE_PERM
    b0 = (bih0 + bhh0)[p].astype(np.float32)
    b1 = (bih1 + bhh1)[p].astype(np.float32)
    # w0x [20, 4, 128]: rows = [x features (19); ones]. Left cols =
    # [Wih0; b0] per gate, right cols = b1 on the ones row.
    # whbig [128, 4, 128]: left cols = [Whh0; 0], right cols =
    # [Wih1; Whh1] -- one K=128 matmul vs hm covers both layers.
    w0x = np.zeros((INPUT + 1, 4, 128), np.float32)
    whbig = np.zeros((128, 4, 128), np.float32)
    for b in range(4):
        w0x[0:INPUT, b, 0:64] = Wih0[p].T[:, b * 64:(b + 1) * 64]
        w0x[INPUT, b, 0:64] = b0[b * 64:(b + 1) * 64]
        w0x[INPUT, b, 64:128] = b1[b * 64:(b + 1) * 64]
        whbig[0:64, b, 0:64] = Whh0[p].T[:, b * 64:(b + 1) * 64]
        whbig[0:64, b, 64:128] = Wih1[p].T[:, b * 64:(b + 1) * 64]
        whbig[64:128, b, 64:128] = Whh1[p].T[:, b * 64:(b + 1) * 64]
    wfcbig = np.zeros((128, 1), np.float32)
    wfcbig[64:128, 0] = Wfc.reshape(HIDDEN)
    base = {
        "w0x": np.ascontiguousarray(w0x.reshape(INPUT + 1, 512)),
        "whbig": np.ascontiguousarray(whbig.reshape(128, 512)),
        "wfc": wfcbig,
        "zeros": np.zeros((128, CB), np.float32),
    }
    xs = x.reshape(NCORES, BL, T, INPUT)
    in_maps = []
    for c in range(NCORES):
        m = dict(base)
        xt = np.empty((T, INPUT + 1, BL), np.float32)
        xt[:, 0:INPUT, :] = xs[c].transpose(1, 2, 0)
        xt[:, INPUT, :] = 1.0
        m["xT"] = xt
        in_maps.append(m)
    return in_maps


_CACHED_NC = None


def kernel(**inputs):
    global _CACHED_NC
    from concourse.bass_utils import run_bass_kernel_spmd

    if _CACHED_NC is None:
        _CACHED_NC = build_nc()
    nc = _CACHED_NC
    in_maps = make_in_maps(**inputs)
    res = run_bass_kernel_spmd(nc, in_maps, list(range(NCORES)))
    outs = [res.results[c]["out"].reshape(BL) for c in range(NCORES)]
    return np.concatenate(outs) + np.float32(inputs["bfc"][0])
